# revision 17
# baseline (speedup 1.0000x reference)
"""CPN detection-head kernel for 8 Trainium2 NeuronCores.

Network: 3x3 conv backbone (3->32) + ReLU, then 4 readout heads
(conv -> scale/shift -> ReLU -> 1x1 conv); score is 3x3, loc/four/ref
are 7x7; ref gets tanh()*3.

Strategy:
  - Shard: batch (2) x 4 row-bands of 192 rows -> 8 cores. Halo rows are
    host-zero-padded; conv zero-padding semantics preserved.
  - All four head convs fused into ONE 7x7 conv with M=128 output
    channels (score's 3x3 weights zero-padded to 7x7), so the PE array
    runs at full output width. The four 1x1 convs fuse into one
    block-diagonal [128 -> 26] matmul.
  - Convs lower to matmuls by replicating the input across partition
    groups with row shifts: K = 4 rows x 32 ch = 128 per matmul; the
    kernel-x shift is a free-dim offset into a column-padded layout.
    7x7 conv = 7 kx * (rows 0-3 K=128 + rows 4-6 K=96) = 14 matmuls.
    Backbone uses full 9-tap im2col (K=27, 1 matmul) over 9 replicas.
  - float32r matmuls: 1 cycle/row at N>=256 (fp32 is 4).
  - Two passes with a DRAM feats scratch: pass 1 backbone -> feats,
    pass 2 reloads feats 4x (DMA does the replication) and runs heads.
"""

import sys

if '/opt/trn_rl_repo' not in sys.path:
    sys.path.insert(0, '/opt/trn_rl_repo')

import numpy as np

import concourse.bass as bass
import concourse.mybir as mybir
import concourse.tile as tile_mod
from concourse import bass_utils
from concourse.tile import TileContext
from concourse.vector_clock import ScopedClock, VectorClock

F32 = mybir.dt.float32
F32R = mybir.dt.float32r
AFT = mybir.ActivationFunctionType

B, CIN, H, W = 2, 3, 768, 768
CB = 32
NCORES = 8
NBANDS = 4              # row-bands per batch
OUT_ROWS = H // NBANDS  # 192
XROWS = OUT_ROWS + 8    # input rows per core (halo 4 each side)
XW = W + 2              # input cols incl 1-col zero pad each side
FROWS = OUT_ROWS + 6    # feats rows per core (halo 3 each side)
P1B = 22                # pass-1 band height (198 = 9 * 22)
HB = 16                 # pass-2 band height (192 = 12 * 16)
WP = W + 6              # column-padded feats width
CHUNK = 384             # matmul moving size (2 chunks per 768 row)

# hidden channel order: [score | loc | four | ref], 32 each
HEADS = ('score', 'loc', 'four', 'ref')
COUT = {'score': 2, 'loc': 2, 'four': 20, 'ref': 2}
COFF = {'score': 0, 'loc': 2, 'four': 4, 'ref': 24}
NOUT = 26
M2 = 34   # 1x1 matmul output partitions: 24 heads + 8 pad + 2 ref (32-aligned)


MAX_WAITS = {}
DEFAULT_MAX_WAITS = 1


def _split_waits_json(bir_json):
    """This walrus build allows very few sync-waits per instruction, but
    Tile attaches up to ~9. Rebalance: move excess waits onto NoOp
    carrier instructions inserted just before, on the same engine.
    Safe under Tile's vector-clock schedule: a wait's condition is
    produced by globally-earlier work only, so stalling the engine
    slightly earlier cannot deadlock."""
    import json as _json
    d = _json.loads(bir_json)
    n = 0
    for f in d['functions']:
        for bb in f['blocks']:
            out = []
            for inst in bb['instructions']:
                si = inst.get('sync_info')
                waits = (si or {}).get('on_wait') or []
                max_waits = MAX_WAITS.get(inst['opcode'], DEFAULT_MAX_WAITS)
                if len(waits) > max_waits:
                    for w in waits[:-max_waits]:
                        n += 1
                        out.append({
                            'debug': inst.get('debug', 0),
                            'engine': inst['engine'],
                            'ins': [], 'outs': [],
                            'name': f'I-WS{n}',
                            'opcode': 'NoOp',
                            'sync_info': {'on_update': [], 'on_wait': [w]},
                        })
                    si['on_wait'] = waits[-max_waits:]
                out.append(inst)
            bb['instructions'] = out
    return _json.dumps(d).encode()


def _patch_compile():
    if getattr(bass_utils, '_split_waits_patched', False):
        return
    orig = bass_utils.compile_bir_kernel

    def patched(bir_json, tmpdir, neff_name='file.neff'):
        return orig(_split_waits_json(bir_json), tmpdir, neff_name)

    bass_utils.compile_bir_kernel = patched
    bass_utils._split_waits_patched = True
    from concourse import bass2jax
    bass2jax.compile_bir_kernel = patched


def _patch_tile_drain():
    """This walrus build rejects >1 sync-wait on a CTRL (Drain) inst.
    Split the tile tail-drain's waits into single-wait drains."""
    if getattr(tile_mod.TileContext, '_drain_split_patched', False):
        return

    def _drain_and_barrier(self, tick_clock, wait_clock):
        nc = self.nc
        gc = tick_clock.global_clock
        n = len(gc)
        for i in range(n):
            if gc[i] > 0:
                v = [0] * n
                v[i] = gc[i]
                d = nc.sync.drain()
                wait_clock.add_sem_waits(d.ins, ScopedClock({None: VectorClock(v)}))
        nc.all_engine_barrier()
        popped = nc._tile_sem_poison_stack.pop()
        assert popped is self._sem_poison
        nc.clear_and_free_semaphores(list(self.sems.allocated().values()))
        nc.all_engine_barrier()

    tile_mod.TileContext._drain_and_barrier = _drain_and_barrier
    tile_mod.TileContext._drain_split_patched = True


def _r(ap):
    return ap  # tiles feeding matmuls are already float32r


def build_nc():
    _patch_tile_drain()
    _patch_compile()
    nc = bass.Bass(trn_type='TRN2', name='cpn')

    x = nc.dram_tensor('x', [CIN, XROWS, XW], F32R, kind='ExternalInput')
    wbb = nc.dram_tensor('wbb', [27, CB], F32R, kind='ExternalInput')
    # per-(channel,row) scale/bias for pass-1 relu: scale is the 0/1
    # row-validity mask (zeroes feats halo rows outside the image, which
    # the reference pads with zeros), bias is bb_b * mask.
    bmask = nc.dram_tensor('bmask', [CB, FROWS], F32, kind='ExternalInput')
    bbias = nc.dram_tensor('bbias', [CB, FROWS], F32, kind='ExternalInput')
    wa = nc.dram_tensor('wa', [7, 128, 128], F32R, kind='ExternalInput')
    wb = nc.dram_tensor('wb', [7, 96, 128], F32R, kind='ExternalInput')
    ball = nc.dram_tensor('ball', [128, 1], F32, kind='ExternalInput')
    w2 = nc.dram_tensor('w2', [128, M2], F32R, kind='ExternalInput')
    b2a = nc.dram_tensor('b2a', [M2, 1], F32, kind='ExternalInput')
    zpad = nc.dram_tensor('zpad', [CB, FROWS + 2, 3], F32R, kind='ExternalInput')
    outs = nc.dram_tensor('outs', [NOUT, OUT_ROWS, W], F32, kind='ExternalOutput')

    with TileContext(nc) as tc:
        with tc.tile_pool(name='dramp', bufs=1, space='DRAM') as dpool:
            # +2 rows so pass-2's uniform replica loads stay in bounds
            feats = dpool.tile([CB, FROWS + 2, WP], F32R)
            # zero the 3 pad columns on each side, once
            nc.sync.dma_start(feats[:, :, 0:3], zpad[:, :, :])
            nc.sync.dma_start(feats[:, :, W + 3:WP], zpad[:, :, :])

            # ---------------- pass 1: backbone ----------------
            with tc.tile_pool(name='p1w', bufs=1) as wpool:
                wbb_sb = wpool.tile([27, CB], F32R)
                nc.sync.dma_start(wbb_sb[:, :], wbb[:, :])
                bmask_sb = wpool.tile_from(bmask[:, :])
                bbias_sb = wpool.tile_from(bbias[:, :])
                with (
                    tc.tile_pool(name='p1x', bufs=2) as xp,
                    tc.tile_pool(name='p1s', bufs=4) as sp,
                    tc.tile_pool(name='p1ps', bufs=4, space='PSUM') as pp,
                ):
                    for band in range(FROWS // P1B):
                        i0 = band * P1B
                        xr = xp.tile([27, P1B, W], F32R)
                        for ky in range(3):
                            for kx in range(3):
                                t = ky * 3 + kx
                                nc.sync.dma_start(
                                    xr[3 * t:3 * t + 3, 0:P1B, 0:W],
                                    x[:, i0 + ky:i0 + ky + P1B, kx:kx + W],
                                )
                        for i in range(P1B):
                            edge = (i0 + i < 3) or (i0 + i >= FROWS - 3)
                            for c0 in (0, CHUNK):
                                ps = pp.tile([CB, CHUNK], F32, tag='ps')
                                nc.tensor.matmul(
                                    ps[:, :], _r(wbb_sb[:, :]),
                                    _r(xr[:, i, c0:c0 + CHUNK]),
                                    start=True, stop=True,
                                )
                                fst = sp.tile([CB, CHUNK], F32R, tag='fst')
                                if edge:
                                    # rows whose mask may be 0: general
                                    # relu(h*mask + bias*mask) on ACT
                                    nc.scalar.activation(
                                        fst[:, :], ps[:, :], AFT.Relu,
                                        bias=bbias_sb[:, i0 + i:i0 + i + 1],
                                        scale=bmask_sb[:, i0 + i:i0 + i + 1],
                                    )
                                else:
                                    # interior: mask==1 -> (h + bias) max 0
                                    nc.vector.tensor_scalar(
                                        fst[:, :], ps[:, :],
                                        bbias_sb[:, i0 + i:i0 + i + 1], 0.0,
                                        mybir.AluOpType.add,
                                        mybir.AluOpType.max,
                                    )
                                nc.gpsimd.dma_start(
                                    feats[:, i0 + i, 3 + c0:3 + c0 + CHUNK],
                                    fst[:, :],
                                )

            # ---------------- pass 2: fused heads ----------------
            with tc.tile_pool(name='p2w', bufs=1) as wp2:
                waA = wp2.tile([128, 7, 128], F32R)
                waB = wp2.tile([96, 7, 128], F32R)
                for kx in range(7):
                    nc.sync.dma_start(waA[:, kx, :], wa[kx])
                    nc.sync.dma_start(waB[:, kx, :], wb[kx])
                w2_sb = wp2.tile([128, M2], F32R)
                nc.sync.dma_start(w2_sb[:, :], w2[:, :])
                ball_sb = wp2.tile_from(ball[:, :])
                b2a_sb = wp2.tile_from(b2a[:, :])
                with (
                    tc.tile_pool(name='p2f', bufs=2) as fp,
                    tc.tile_pool(name='p2h', bufs=4) as hp,
                    tc.tile_pool(name='p2o', bufs=6) as op,
                    tc.tile_pool(name='psh', bufs=4, space='PSUM') as pph,
                    tc.tile_pool(name='pso', bufs=3, space='PSUM') as ppo,
                ):
                    # the 1x1 + epilogue for a chunk is emitted one chunk
                    # late, so the PE never stalls waiting for the relu:
                    # its stream is [14xMM chunk k][1x1 chunk k-1][14xMM
                    # chunk k+1][1x1 chunk k]...
                    pending = None

                    def flush(p):
                        hid_p, ob_p, obr_p, row_p, c0_p = p
                        po = ppo.tile([M2, CHUNK], F32, tag='po')
                        nc.tensor.matmul(
                            po[:, :], _r(w2_sb[:, :]), _r(hid_p[:, :]),
                            start=True, stop=True,
                        )
                        nc.vector.tensor_scalar_add(
                            ob_p[0:24, c0_p:c0_p + CHUNK], po[0:24, :],
                            b2a_sb[0:24, 0:1],
                        )
                        nc.scalar.activation(
                            obr_p[0:2, c0_p:c0_p + CHUNK], po[32:34, :],
                            AFT.Tanh, bias=b2a_sb[32:34, 0:1], scale=1.0,
                        )
                        nc.vector.tensor_scalar_mul(
                            obr_p[0:2, c0_p:c0_p + CHUNK],
                            obr_p[0:2, c0_p:c0_p + CHUNK], 3.0,
                        )
                        if c0_p == CHUNK:
                            nc.gpsimd.dma_start(
                                outs[0:24, row_p, :], ob_p[:, :])
                            nc.gpsimd.dma_start(
                                outs[24:26, row_p, :], obr_p[:, :])

                    for band in range(OUT_ROWS // HB):
                        y0 = band * HB
                        f4 = fp.tile([128, HB + 4, WP], F32R)
                        for j in range(4):
                            nc.sync.dma_start(
                                f4[32 * j:32 * j + 32, 0:HB + 4, 0:WP],
                                feats[:, y0 + j:y0 + j + HB + 4, 0:WP],
                            )
                        for t in range(HB):
                            ob = op.tile([24, W], F32, tag='ob')
                            obr = op.tile([2, W], F32, tag='obr')
                            for c0 in (0, CHUNK):
                                ph = pph.tile([128, CHUNK], F32, tag='ph')
                                for kx in range(7):
                                    nc.tensor.matmul(
                                        ph[:, :], _r(waA[:, kx, :]),
                                        _r(f4[:, t, c0 + kx:c0 + kx + CHUNK]),
                                        start=(kx == 0), stop=False,
                                    )
                                for kx in range(7):
                                    nc.tensor.matmul(
                                        ph[:, :], _r(waB[:, kx, :]),
                                        _r(f4[0:96, t + 4, c0 + kx:c0 + kx + CHUNK]),
                                        start=False, stop=(kx == 6),
                                    )
                                hid = hp.tile([128, CHUNK], F32R, tag='hid')
                                # sall is folded into wa/wb host-side:
                                # relu = (h + ball) max 0, on the DVE
                                nc.vector.tensor_scalar(
                                    hid[:, :], ph[:, :],
                                    ball_sb[:, 0:1], 0.0,
                                    mybir.AluOpType.add,
                                    mybir.AluOpType.max,
                                )
                                if pending is not None:
                                    flush(pending)
                                pending = (hid, ob, obr, y0 + t, c0)
                    flush(pending)
    return nc


def _prep_weights(inp):
    g = {k: np.asarray(v, np.float32) for k, v in inp.items()}

    bb_w = g['bb_w']  # [32, 3, 3, 3]
    wbb = np.ascontiguousarray(
        bb_w.transpose(2, 3, 1, 0).reshape(27, CB))           # [(ky,kx,c), m]
    bbb = g['bb_b']

    wfull = np.zeros((128, CB, 7, 7), np.float32)
    for hi, h in enumerate(HEADS):
        w1 = g[h + '_w1']
        if h == 'score':
            wfull[hi * 32:(hi + 1) * 32, :, 2:5, 2:5] = w1
        else:
            wfull[hi * 32:(hi + 1) * 32] = w1
    t = wfull.transpose(2, 3, 1, 0)                            # [ky, kx, c, m]
    wa = np.ascontiguousarray(
        t[0:4].transpose(1, 0, 2, 3).reshape(7, 128, 128))     # [kx, (j,c), m]
    wb = np.ascontiguousarray(
        t[4:7].transpose(1, 0, 2, 3).reshape(7, 96, 128))

    sall = np.concatenate([g[h + '_s'] for h in HEADS])
    wa = np.ascontiguousarray(wa * sall[None, None, :])
    wb = np.ascontiguousarray(wb * sall[None, None, :])
    ball = np.concatenate(
        [g[h + '_b1'] * g[h + '_s'] + g[h + '_t'] for h in HEADS]).reshape(128, 1)

    w2 = np.zeros((128, M2), np.float32)
    b2a = np.zeros((M2, 1), np.float32)
    for hi, h in enumerate(HEADS):
        blk = g[h + '_w2'][:, :, 0, 0]                         # [cout, 32]
        off = COFF[h] if h != 'ref' else 32
        w2[hi * 32:(hi + 1) * 32, off:off + COUT[h]] = blk.T
        b2a[off:off + COUT[h], 0] = g[h + '_b2']

    return dict(wbb=wbb, wa=wa, wb=wb, ball=ball,
                w2=w2, b2a=b2a), bbb


_NC_CACHE = None


def _get_nc():
    global _NC_CACHE
    if _NC_CACHE is None:
        _NC_CACHE = build_nc()
    return _NC_CACHE


def run_shards(inputs, trace=False):
    """Build in_maps, run on 8 cores, return (results, BassKernelResults)."""
    shared, bbb = _prep_weights(inputs)
    xin = np.asarray(inputs['inputs'], np.float32)
    xpad = np.zeros((B, CIN, H + 8, XW), np.float32)
    xpad[:, :, 4:4 + H, 1:1 + W] = xin

    in_maps = []
    for c in range(NCORES):
        b, band = divmod(c, NBANDS)
        r0 = band * OUT_ROWS
        x_core = np.ascontiguousarray(xpad[b, :, r0:r0 + XROWS, :])
        m = np.ones(FROWS, np.float32)
        for i in range(FROWS):
            if not (0 <= r0 - 3 + i < H):
                m[i] = 0.0
        bmask_c = np.broadcast_to(m, (CB, FROWS)).copy()
        bbias_c = bbb[:, None] * m[None, :]
        in_maps.append(dict(shared, x=x_core,
                            bmask=np.ascontiguousarray(bmask_c, np.float32),
                            bbias=np.ascontiguousarray(bbias_c, np.float32),
                            zpad=np.zeros((CB, FROWS + 2, 3), np.float32)))

    nc = _get_nc()
    res = bass_utils.run_bass_kernel_spmd(
        nc, in_maps, core_ids=list(range(NCORES)), trace=trace)

    scores = np.empty((B, 2, H, W), np.float32)
    locations = np.empty((B, 2, H, W), np.float32)
    refinement = np.empty((B, 2, H, W), np.float32)
    fourier = np.empty((B, 20, H, W), np.float32)
    for c in range(NCORES):
        b, band = divmod(c, NBANDS)
        r0 = band * OUT_ROWS
        o = res.results[c]['outs']
        scores[b, :, r0:r0 + OUT_ROWS] = o[0:2]
        locations[b, :, r0:r0 + OUT_ROWS] = o[2:4]
        fourier[b, :, r0:r0 + OUT_ROWS] = o[4:24]
        refinement[b, :, r0:r0 + OUT_ROWS] = o[24:26]
    return (scores, locations, refinement, fourier), res


def kernel(**inputs):
    out, _ = run_shards(inputs, trace=False)
    return out

